# revision 1
# baseline (speedup 1.0000x reference)
"""Trainium2 Bass kernel for nn_Encoder_Flows (3-layer dense GCN message passing).

Math per graph (reference):
    A = flows [N, N];  deg[c] = sum_r A[r, c];  dinv = rsqrt(deg)
    L(x, W, b) = dinv * (A^T @ (dinv * (x @ W))) + b
    out = L(L(L(A, W1, b1), W2, b2), W3, b3)          # [N, 128]

Key algebraic fusion (bias-free case, which is what setup_inputs produces):
with M = diag(dinv) A^T diag(dinv), node-dim M commutes with feature-dim W:
    out = M^3 (A @ W1) @ (W2 @ W3)
i.e. four dense [N,N]x[N,128] matmuls + a tiny 128x128 epilogue, instead of
A-matmuls over 640 feature columns. Nonzero biases are handled exactly by a
host-side rank-1 correction (m2 (x) b1W2W3 + m1 (x) b2W3 + 1 (x) b3).

All four big matmuls run as fp8e4m3 DoubleRow (K packed 2 per partition; at
the full 2.4GHz p-state a 512-col DoubleRow matmul retires in ~219ns = 2x
the bf16 MAC rate). fp8 quantization noise in the *node-mean* direction is
amplified ~sqrt(N)x by the adjacency's Perron mode, so plain fp8
intermediates fail; three cheap exact corrections fix that:
  - W1 is quantized on the host with per-column dithering (entries bumped
    one fp8 level so each column's quantization error sums to ~0).
  - w1's error column-mean mu1 is measured on device (eps matmul against a
    ones vector); since rdeg*deg == 1 the rank-1 correction deg (x) mu1 on
    t2 collapses to "add mu1 to every row of w2" -- fused into w2's
    production.
  - w2's mu2 is applied as an explicit K=1 rank-1 matmul deg' (x) mu2
    accumulated into t3's PSUM.
Scale folds: A*2^7, W1*2^10, w*2^8, W23*2^-15 keep everything in fp8 range.

Host-side preparation (all O(N^2) data prep, matching the quantized A the
device multiplies with): fp8 quantize + DoubleRow packing of A and A^T,
dithered W1, W2@W3, and the degree-normalization vectors
(dinv/dinv0/rdeg1 node-major + deg' as a bf16 row). All O(N^2 * D) matmul
work runs on device.

The schedule keeps the tensor engine's in-order queue gap-free (a bubble
drops the PE p-state from 2.4GHz to 1.2GHz for ~3us): graph 0's first
M-apply runs while graph 1's inputs stream, later M-applies interleave the
two graphs chunk-by-chunk, each chunk's transposes trail one slot behind
its matmuls, and the epilogue is fused into the last M-apply's chunk loop.
"""

import sys
from contextlib import ExitStack

import numpy as np

for _p in ("/opt/trn_rl_repo", "/opt/pypackages"):
    if _p not in sys.path:
        sys.path.append(_p)

import ml_dtypes

B, N, P = 16, 2048, 128
NB = N // P          # 16 row/col 128-blocks
KB = N // (2 * P)    # 8 DoubleRow pair-blocks (256 rows each)
NCORES = 8
GPC = B // NCORES    # graphs per core
D = 128              # feature width carried through the fused chain
CH = 512             # psum chunk (one bank of fp32)
NCH = N // CH

_COMPILED = {}


def _build():
    import concourse.mybir as mybir
    import concourse.tile as tile
    from concourse import bacc

    f32 = mybir.dt.float32
    bf16 = mybir.dt.bfloat16
    fp8 = mybir.dt.float8e4
    DR = mybir.MatmulPerfMode.DoubleRow
    MUL = mybir.AluOpType.mult
    ADD = mybir.AluOpType.add
    SUB = mybir.AluOpType.subtract
    COPY = mybir.ActivationFunctionType.Copy

    nc = bacc.Bacc("TRN2", target_bir_lowering=False)
    A8_d = nc.declare_dram_parameter("A8", [GPC, KB, P, NCH, 2, CH], fp8,
                                     isOutput=False)
    A8t_d = nc.declare_dram_parameter("A8t", [GPC, KB, P, NCH, 2, CH], fp8,
                                      isOutput=False)
    W1p_d = nc.declare_dram_parameter("W1p", [KB, P, 2, D], fp8, isOutput=False)
    W23_d = nc.declare_dram_parameter("W23", [P, D], bf16, isOutput=False)
    DN_d = nc.declare_dram_parameter("DN", [GPC, 3, P, NB], f32, isOutput=False)
    DR_d = nc.declare_dram_parameter("DROW", [GPC, 1, N], bf16, isOutput=False)
    IOB_d = nc.declare_dram_parameter("IOB", [P, P], bf16, isOutput=False)
    out_d = nc.declare_dram_parameter("out", [GPC, N, D], f32, isOutput=True)

    with tile.TileContext(nc) as tc, ExitStack() as ctx:
        # PSUM budget (8 banks): pu = 2 tags (U-phase chunks 0,1) + pt = 4
        # tags (U chunks 2,3 / T-phase chunks, per graph pair) + ptr = 2 tags
        # x 1 buf (mu-path psum + f32 epilogue quads).
        wpool = ctx.enter_context(tc.tile_pool(name="wpool", bufs=1))
        spool = ctx.enter_context(tc.tile_pool(name="spool", bufs=1))
        apool = ctx.enter_context(tc.tile_pool(name="apool", bufs=1))
        dpool = ctx.enter_context(tc.tile_pool(name="dpool", bufs=1))
        cpool = ctx.enter_context(tc.tile_pool(name="cpool", bufs=3))
        qpool = ctx.enter_context(tc.tile_pool(name="qpool", bufs=2))
        wqp = ctx.enter_context(tc.tile_pool(name="wqp", bufs=1))
        epool = ctx.enter_context(tc.tile_pool(name="epool", bufs=1))
        mpool = ctx.enter_context(tc.tile_pool(name="mpool", bufs=1))
        ogp = ctx.enter_context(tc.tile_pool(name="ogp", bufs=2))
        pu = ctx.enter_context(tc.tile_pool(name="pu", bufs=1, space="PSUM"))
        pt = ctx.enter_context(tc.tile_pool(name="pt", bufs=1, space="PSUM"))
        ptr = ctx.enter_context(tc.tile_pool(name="ptr", bufs=1, space="PSUM"))

        # --- weights, identities, ones (replicated across cores) ---
        W1p = wpool.tile([P, KB, 2, D], fp8)
        nc.sync.dma_start(W1p[:], W1p_d.ap().rearrange("kb p i d -> p kb i d"))
        W23 = wpool.tile([P, D], bf16)
        nc.sync.dma_start(W23[:], W23_d.ap())
        iob = wpool.tile([P, P], bf16)
        nc.sync.dma_start(iob[:], IOB_d.ap())
        onesb = wpool.tile([P, 1], bf16)
        nc.vector.memset(onesb[:], 1.0)
        onesrow = wpool.tile([1, P], bf16)
        nc.vector.memset(onesrow[:], 1.0)

        out_ap = out_d.ap().rearrange("g (qb p) d -> g p qb d", p=P)

        A8 = {}       # natural packed A, resident for the 3 M-applies
        strips = {}   # A^T packed strips (U phase)
        w8 = {}       # current fp8 lhsT per graph
        dn = {}       # [P, 3, NB]: dinv / dinv*2^-9 / rdeg*2^-7
        degrow = {}   # 2^7 deg as bf16 [1, N] (rank-1 rhs)
        murep = {}    # mu1 replicated [P, D] bf16
        mu2row = {}   # mu2 [1, D] bf16
        eps = {}      # quantization residual, node-major bf16

        def load_g(g):
            strips[g] = []
            for cb in range(KB):
                st = spool.tile([P, NCH, 2, CH], fp8, tag=f"st{cb}",
                                name=f"st{g}_{cb}")
                nc.sync.dma_start(st[:], A8t_d.ap()[g][cb])
                strips[g].append(st)
            A8[g] = apool.tile([P, KB, NCH, 2, CH], fp8, tag=f"A{g}",
                               name=f"A8_{g}")
            for kb in range(KB):
                nc.sync.dma_start(A8[g][:, kb], A8_d.ap()[g][kb])
            dnt = dpool.tile([P, 3, NB], f32, tag=f"dn{g}", name=f"dn{g}")
            nc.sync.dma_start(dnt[:], DN_d.ap()[g].rearrange("k p nb -> p k nb"))
            dn[g] = dnt
            drw = mpool.tile([1, N], bf16, tag=f"degrow{g}", name=f"degrow{g}")
            nc.sync.dma_start(drw[:], DR_d.ap()[g])
            degrow[g] = drw

        def utag(g, ch):
            return (pu, f"u{ch}") if ch < 2 else (pt, f"p{2 * g + ch - 2}")

        def phase_U(g):
            """u' = (A*2^7) @ (W1*2^10) via A^T strips, strip-paced (cb outer
            so each arriving strip feeds 4 matmuls immediately)."""
            ups = []
            for ch in range(NCH):
                pool, tag = utag(g, ch)
                ups.append(pool.tile([P, CH], f32, tag=tag, name=f"ups{g}{ch}"))
            for cb in range(KB):
                for ch in range(NCH):
                    nc.tensor.matmul(ups[ch][:], W1p[:, cb],
                                     strips[g][cb][:, ch],
                                     start=(cb == 0), stop=(cb == KB - 1),
                                     perf_mode=DR)
            w = wqp.tile([P, NB, D], fp8, tag=f"w0{g}", name=f"w0{g}")
            for ch in range(NCH):
                ub = cpool.tile([P, CH], bf16, tag=f"cb{g}", name=f"ub{g}{ch}")
                nc.vector.tensor_copy(ub[:], ups[ch][:])
                tq = ptr.tile([P, 4, P], bf16, tag=f"tr{g}", name=f"utq{g}{ch}")
                for j in range(4):
                    nc.tensor.transpose(tq[:, j], ub[:, j * P:(j + 1) * P], iob[:])
                sl = slice(ch * 4, ch * 4 + 4)
                nc.vector.tensor_tensor(
                    w[:, sl], tq[:],
                    dn[g][:, 1, sl, None].to_broadcast([P, 4, D]), MUL)
            w8[g] = w

        # ---- M-apply phases ----

        def emit_mms(g, step, ch):
            tps = pt.tile([P, CH], f32, tag=f"p{2 * g + ch % 2}",
                          name=f"tps{g}{step}{ch}")
            for kb in range(KB):
                nc.tensor.matmul(tps[:], w8[g][:, 2 * kb:2 * kb + 2],
                                 A8[g][:, kb, ch],
                                 start=(kb == 0),
                                 stop=(kb == KB - 1 and step != 3),
                                 perf_mode=DR)
            if step == 3:
                nc.tensor.matmul(tps[:], mu2row[g],
                                 degrow[g][:, ch * CH:(ch + 1) * CH],
                                 start=False, stop=True, skip_group_check=True)
            return tps

        def emit_post(g, step, ch, tps, wn, ep):
            sl = slice(ch * 4, ch * 4 + 4)
            if step == 3:
                # fused epilogue: retire chunk (ACT), 4 mms vs W23, scale, out
                tc3 = cpool.tile([P, CH], bf16, tag=f"c3{g}", name=f"tc3{g}{ch}")
                nc.scalar.activation(tc3[:], tps[:], COPY)
                epq = ptr.tile([P, 4, P], f32, tag=f"tr{g}", name=f"epq{g}{ch}")
                for j in range(4):
                    nc.tensor.matmul(epq[:, j], tc3[:, j * P:(j + 1) * P],
                                     W23[:], start=True, stop=True)
                og = ogp.tile([P, 4, D], f32, tag=f"og{g}", name=f"og{g}{ch}")
                nc.vector.tensor_tensor(
                    og[:], epq[:],
                    dn[g][:, 0, sl, None].to_broadcast([P, 4, D]), MUL)
                nc.sync.dma_start(out_ap[g][:, sl], og[:])
                return
            tf = cpool.tile([P, CH], bf16, tag=f"cb{g}", name=f"tf{g}{step}{ch}")
            nc.vector.tensor_copy(tf[:], tps[:])
            tq = ptr.tile([P, 4, P], bf16, tag=f"tr{g}", name=f"ttq{g}{step}{ch}")
            for j in range(4):
                nc.tensor.transpose(tq[:, j], tf[:, j * P:(j + 1) * P], iob[:])
            if step == 1:
                wex = qpool.tile([P, 4, D], bf16, tag=f"wx{g}", name=f"wx{g}{ch}")
                nc.vector.tensor_tensor(
                    wex[:], tq[:],
                    dn[g][:, 2, sl, None].to_broadcast([P, 4, D]), MUL)
            else:  # step 2: + mu1 broadcast
                wt = qpool.tile([P, 4, D], bf16, tag=f"wt{g}", name=f"wt{g}{ch}")
                nc.vector.tensor_tensor(
                    wt[:], tq[:],
                    dn[g][:, 2, sl, None].to_broadcast([P, 4, D]), MUL)
                wex = qpool.tile([P, 4, D], bf16, tag=f"wx{g}", name=f"wx2{g}{ch}")
                nc.vector.tensor_tensor(
                    wex[:], wt[:], murep[g][:, None, :].to_broadcast([P, 4, D]),
                    ADD)
            nc.scalar.activation(wn[:, sl], wex[:], COPY)
            nc.vector.tensor_tensor(ep[:, sl], wex[:], wn[:, sl], SUB)

        step_w, step_e = {}, {}

        def run_slots(step, slots):
            for g, _ in slots:
                if step != 3 and (step, g) not in step_w:
                    step_w[(step, g)] = wqp.tile(
                        [P, NB, D], fp8, tag=f"w{step}{g}", name=f"w{step}{g}")
                    step_e[(step, g)] = epool.tile(
                        [P, NB, D], bf16, tag=f"eps{g}", name=f"eps{step}{g}")
            pend = []
            for g, ch in slots:
                tps = emit_mms(g, step, ch)
                pend.append((g, ch, tps))
                if len(pend) > 1:
                    pg, pch, ptps = pend.pop(0)
                    emit_post(pg, step, pch, ptps,
                              step_w.get((step, pg)), step_e.get((step, pg)))
            for pg, pch, ptps in pend:
                emit_post(pg, step, pch, ptps,
                          step_w.get((step, pg)), step_e.get((step, pg)))

        def finish_g(step, g):
            w8[g], eps[g] = step_w[(step, g)], step_e[(step, g)]

        def phase_MU(g, step):
            """mu = col-mean of eps (x 2^-11 fold); step1 -> replicated
            [P, D] bf16; step2 -> [1, D] bf16 row for the rank-1 matmul."""
            muT = ptr.tile([P, 1], f32, tag=f"tr{g}", name=f"muT{g}{step}")
            for nb in range(NB):
                nc.tensor.matmul(muT[:], eps[g][:, nb], onesb[:],
                                 start=(nb == 0), stop=(nb == NB - 1))
            muTs = mpool.tile([P, 1], bf16, tag=f"muTs{g}", name=f"muTs{g}{step}")
            nc.vector.tensor_scalar_mul(muTs[:], muT[:], 2.0 ** -11)
            rowp = ptr.tile([1, P], bf16, tag=f"tr{g}", name=f"murp{g}{step}")
            nc.tensor.transpose(rowp[:], muTs[:], iob[:])
            row = mpool.tile([1, P], bf16, tag=f"mur{g}_{step}", name=f"mur{g}{step}")
            nc.vector.tensor_copy(row[:], rowp[:])
            if step == 1:
                repp = ptr.tile([P, P], f32, tag=f"tr{g}", name=f"repp{g}")
                nc.tensor.matmul(repp[:], onesrow[:], row[:], start=True, stop=True)
                rep = mpool.tile([P, D], bf16, tag=f"murep{g}", name=f"murep{g}")
                nc.vector.tensor_copy(rep[:], repp[:])
                murep[g] = rep
            else:
                mu2row[g] = row

        # ---- schedule: T1(g0) overlaps graph 1's input streaming ----
        # graph 0's whole chain needs no new HBM data after its A8 lands, so
        # it runs front-loaded while graph 1's 8MB streams; graph 1's chain
        # follows with everything resident. U(1) sits after T2(0) so its
        # strip-paced matmuls don't head-of-line-block ready work.
        load_g(0)
        load_g(1)
        phase_U(0)
        run_slots(1, [(0, ch) for ch in range(NCH)])
        finish_g(1, 0)
        phase_MU(0, 1)
        run_slots(2, [(0, ch) for ch in range(NCH)])
        finish_g(2, 0)
        phase_U(1)
        phase_MU(0, 2)
        run_slots(3, [(0, ch) for ch in range(NCH)])
        run_slots(1, [(1, ch) for ch in range(NCH)])
        finish_g(1, 1)
        phase_MU(1, 1)
        run_slots(2, [(1, ch) for ch in range(NCH)])
        finish_g(2, 1)
        phase_MU(1, 2)
        run_slots(3, [(1, ch) for ch in range(NCH)])

    nc.compile()
    return nc


def _get_nc():
    if "nc" not in _COMPILED:
        _COMPILED["nc"] = _build()
    return _COMPILED["nc"]


FP8 = ml_dtypes.float8_e4m3


def _q8(x):
    return np.clip(x, -240.0, 240.0).astype(FP8)


def _dither_q8(xs):
    """Per-column fp8 quantization with near-zero column error means: greedily
    bump the entries whose residual leans furthest toward the column's error
    direction by one representable level until the mean is cancelled."""
    q = _q8(xs)
    qf = q.astype(np.float32)
    r = xs - qf
    m = r.sum(0)
    s = np.where(m >= 0, 1.0, -1.0).astype(np.float32)
    u = q.view(np.uint8)
    mag = (u & 0x7F).astype(np.int16)
    neg = (u & 0x80) != 0
    dirpos = np.broadcast_to(s > 0, xs.shape)
    away = (~neg) == dirpos
    nmag = np.where(mag == 0, 1, np.where(away, mag + 1, mag - 1))
    nsign = np.where(mag == 0, ~dirpos, neg)
    nb = ((nmag.astype(np.uint8) & 0x7F) | (nsign.astype(np.uint8) << 7))
    nxt = nb.view(FP8).astype(np.float32)
    ok = np.isfinite(nxt) & (np.abs(nxt) <= 240.0) & (nmag <= 0x7E)
    step = np.where(ok, nxt - qf, 0.0)
    key = np.where(ok, r * s[None, :], -np.inf)
    order = np.argsort(-key, axis=0)
    step_sorted = np.take_along_axis(step, order, axis=0)
    cum = np.cumsum(step_sorted, axis=0)
    err = np.abs(m[None, :] - cum)
    k = np.argmin(np.vstack([np.abs(m)[None, :], err]), axis=0)  # 0 = no bumps
    out = qf.copy()
    for d in range(xs.shape[1]):
        if k[d] > 0:
            idx = order[:k[d], d]
            out[idx, d] = nxt[idx, d]
    return out.astype(FP8)


def _pack_pairs(x):
    """[..., n*256, F] -> [..., n, 128, 2, F] DoubleRow packing (K pairs)."""
    s = x.shape
    y = x.reshape(s[:-2] + (s[-2] // 256, 2, P, s[-1]))
    return np.ascontiguousarray(np.swapaxes(y, -3, -2))


def kernel(flows, W1, b1, W2, b2, W3, b3, _trace=False):
    from concourse.bass_utils import run_bass_kernel_spmd

    flows = np.asarray(flows, dtype=np.float32)
    W1 = np.asarray(W1, dtype=np.float32)
    W2 = np.asarray(W2, dtype=np.float32)
    W3 = np.asarray(W3, dtype=np.float32)
    b1 = np.asarray(b1, dtype=np.float32)
    b2 = np.asarray(b2, dtype=np.float32)
    b3 = np.asarray(b3, dtype=np.float32)

    nc = _get_nc()

    def _chunk(x):
        """[B, KB, P, 2, N] -> [B, KB, P, NCH, 2, CH] (contiguous rhs slices)."""
        return np.ascontiguousarray(
            x.reshape(B, KB, P, 2, NCH, CH).transpose(0, 1, 2, 4, 3, 5))

    A8 = _q8(flows * 2.0 ** 7)                       # [B, N, N] fp8, value 2^7 A
    A8n = _chunk(_pack_pairs(A8))
    A8t = _chunk(_pack_pairs(np.ascontiguousarray(A8.transpose(0, 2, 1))))

    W1p = _pack_pairs(_dither_q8(W1 * 2.0 ** 10))    # [KB, P, 2, D]
    W23 = ((W2 @ W3) * 2.0 ** -15).astype(ml_dtypes.bfloat16)

    # degree-normalization vectors from the quantized A the device uses
    degs = A8.astype(np.float32).sum(axis=1)         # [B, N] = 2^7 * deg
    dinv = (1.0 / np.sqrt(degs * 2.0 ** -7)).astype(np.float32)
    dinv0 = dinv * np.float32(2.0 ** -9)
    rdeg1 = (1.0 / degs).astype(np.float32)          # rdeg * 2^-7
    DN = np.stack([dinv, dinv0, rdeg1], axis=1)      # [B, 3, N]
    DN = np.ascontiguousarray(
        DN.reshape(B, 3, NB, P).transpose(0, 1, 3, 2))  # node-major [B,3,P,NB]
    DROW = degs.astype(ml_dtypes.bfloat16).reshape(B, 1, N)

    in_maps = []
    for c in range(NCORES):
        sl = slice(c * GPC, (c + 1) * GPC)
        in_maps.append({
            "A8": A8n[sl], "A8t": A8t[sl],
            "W1p": W1p, "W23": W23,
            "DN": DN[sl], "DROW": DROW[sl],
            "IOB": np.eye(P, dtype=ml_dtypes.bfloat16),
        })

    res = run_bass_kernel_spmd(nc, in_maps, core_ids=list(range(NCORES)), trace=_trace)
    out = np.concatenate([res.results[c]["out"] for c in range(NCORES)], axis=0)
    out = np.ascontiguousarray(out.astype(np.float32))

    if np.any(b1) or np.any(b2) or np.any(b3):
        # exact bias terms (see module docstring): rank-1 in the node dim,
        # computed on host from the full-precision A
        deg = flows.sum(axis=1)                      # [B, N] column sums
        dv = np.where(deg > 0, 1.0 / np.sqrt(deg), 0.0).astype(np.float32)
        m1 = dv * np.einsum('brc,br->bc', flows, dv)
        m2 = dv * np.einsum('brc,br->bc', flows, dv * m1)
        out += m2[..., None] * (b1 @ W2 @ W3)[None, None, :]
        out += m1[..., None] * (b2 @ W3)[None, None, :]
        out += b3[None, None, :]

    if _trace:
        return out, res
    return out

